# revision 47
# baseline (speedup 1.0000x reference)
"""MoE + LoRA expert FFN kernel for 8 Trainium2 NeuronCores.

Strategy (expert-parallel, host dispatch/combine):
  - E=8 experts, one expert per core. The host groups tokens by expert
    (duplicate selections collapse with summed routing weight), pads each
    group to a uniform capacity C (multiple of 8 — NOT 128: moving dims
    are arbitrary, so padding to the max expert load + 8 saves ~4% PE
    time vs ceil-to-128), and ships per-core inputs packed so that every
    DMA is one trigger with 128 contiguous per-partition runs (2-8 KB
    descriptors — small strided descriptors measured ~15 GB/s, killing
    startup):
        xg  [128, KH*C]   token-tile-major:  [ti][k][t]
        wg  [128, KH*I]   column-group-major:[q][k][c]   (gate+LoRA)
        wu  [128, KH*I]   same for up+LoRA
        wd  [128, KI*H]   i-chunk-major:     [i][c]      (down+LoRA)
    and receives yT [H, C] fp32 = (silu(x@Wg) * (x@Wu)) @ Wd, transposed.
  - Matmuls run bf16 (storage + PE input; fp32 PSUM accumulate).
    Measured rel err 4.3e-3 vs the 2e-2 gate. bf16 enables FWL (fast
    weight load) and halves DMA vs fp32r.
  - The host scales each token's expert output by its routing weight and
    scatters back into the [T, H] result.

LoRA folding is exact algebra: x@W + s*(x@A)@B == x@(W + s*A@B).
"""

import numpy as np
import ml_dtypes

E, H, I, R, TOPK = 8, 1024, 2816, 8, 2
SCALING = 2.0
NCORES = 8
KP = 128          # partition / contraction tile
KH = H // KP      # 8 contraction chunks over H
KI = I // KP      # 22 chunks over I
HH = H // KP      # 8 output row blocks
NTOK = 512        # moving-dim (token) tile
DUMMIES = 11      # PE warm-up matmuls bridging the first weight DMAs
BF16 = ml_dtypes.bfloat16

_cache = {}


def _setup_paths():
    import sys
    for p in ("/opt/trn_rl_repo", "/root/.axon_site"):
        if p not in sys.path:
            sys.path.insert(0, p)


def _split_multi_waits(nc):
    """The walrus in this container accepts at most 1 sem wait per
    instruction (2 on EventSemaphore); Tile emits more. Rewrite each block,
    moving excess waits onto preceding single-wait NoOps on the same
    engine (engines execute in order, so semantics are preserved)."""
    _setup_paths()
    from bass_rust import SyncInfo
    from concourse import mybir

    ctr = [0]
    for f in nc.m.functions:
        for bb in f.blocks:
            insts = bb.instructions
            new = []
            changed = False
            for inst in insts:
                si = inst.sync_info
                waits = list(si.on_wait or []) if si is not None else []
                cap = 2 if isinstance(inst, mybir.InstEventSemaphore) else 1
                if len(waits) > cap:
                    changed = True
                    for w in waits[:-cap]:
                        nop = mybir.InstNoOp(
                            name=f"SW-{ctr[0]}", ins=[], outs=[])
                        ctr[0] += 1
                        nop.engine = inst.engine
                        nop.sync_info = SyncInfo(on_wait=[w], on_update=[])
                        new.append(nop)
                    inst.sync_info = SyncInfo(
                        on_wait=waits[-cap:],
                        on_update=list(si.on_update or []))
                new.append(inst)
            if changed:
                bb.instructions = new


def _token_tiles(C):
    tiles = []
    t0 = 0
    while t0 < C:
        tw = min(NTOK, C - t0)
        tiles.append((t0, tw))
        t0 += tw
    return tiles


# ramped weight column groups (in i-tiles): small first for fast start
W_GROUPS = [1, 1, 2, 2]
while sum(W_GROUPS) < KI:
    W_GROUPS.append(min(4, KI - sum(W_GROUPS)))
W_GSTART = [sum(W_GROUPS[:j]) for j in range(len(W_GROUPS))]
NG = len(W_GROUPS)
# per-tag slot counts sized from the alive-interval overlap under the
# "load q+2 at first i of group q" prefetch policy (wg+wu share the tag)
W_BUFS = {1: 4, 2: 4, 4: 6}

# wd chunk sizes (in i-tiles); chunks 0/1 prefetched during phase B
WD_CHUNKS = [3, 4, 4, 4, 4, 3]
WD_CSTART = [sum(WD_CHUNKS[:j]) for j in range(len(WD_CHUNKS))]
assert sum(WD_CHUNKS) == KI


def _build(C):
    """Build the per-core Bass program for token capacity C (mult of 8)."""
    _setup_paths()
    import concourse.bass as bass
    import concourse.tile as tile
    from concourse import mybir

    f32 = mybir.dt.float32
    sdt = mybir.dt.bfloat16

    nc = bass.Bass("TRN2", target_bir_lowering=False, debug=False,
                   num_devices=NCORES)
    xg = nc.declare_dram_parameter("xg", [KP, KH * C], sdt, isOutput=False)
    wg = nc.declare_dram_parameter("wg", [KP, KH * I], sdt, isOutput=False)
    wu = nc.declare_dram_parameter("wu", [KP, KH * I], sdt, isOutput=False)
    wd = nc.declare_dram_parameter("wd", [KP, KI * H], sdt, isOutput=False)
    yT = nc.declare_dram_parameter("yT", [H, C], f32, isOutput=True)

    ttiles = _token_tiles(C)
    NT = len(ttiles)

    i2q = {}
    for qq, (g0, gn) in enumerate(zip(W_GSTART, W_GROUPS)):
        for i in range(g0, g0 + gn):
            i2q[i] = (qq, i - g0)
    i2c = {}
    for cc, (c0, cn) in enumerate(zip(WD_CSTART, WD_CHUNKS)):
        for i in range(c0, c0 + cn):
            i2c[i] = (cc, i - c0)

    with tile.TileContext(nc) as tc:
        with tc.tile_pool(name="hh", bufs=1) as hp, \
             tc.tile_pool(name="wdp", bufs=1) as wdp:
            h_t = [hp.tile([KP, C], sdt, tag=f"h{i}", name=f"h{i}")
                   for i in range(KI)]
            wd_t = {}     # chunk idx -> [KP, cn*H] tile

            def load_wd_chunk(c):
                # rides the SP HWDGE ring BEHIND the wg groups: ring FIFO
                # keeps the 5.5MB of wd traffic out of the startup HBM
                # crunch (HBM ~358GB/s/core is the startup bottleneck;
                # an idle queue would fire all chunks at t=0 and starve
                # the first wg/wu/x arrivals)
                cn = WD_CHUNKS[c]
                t = wdp.tile([KP, cn * H], sdt, tag=f"wdc{c}",
                             name=f"wdc{c}")
                o = WD_CSTART[c] * H
                nc.sync.dma_start(out=t, in_=wd[:, o:o + cn * H])
                wd_t[c] = t

            # ---- phase B: h = silu(x@wg) * (x@wu), feature-major [I, C]
            with tc.tile_pool(name="xp", bufs=1) as xp, \
                 tc.tile_pool(name="wst", bufs=1) as wst, \
                 tc.tile_pool(name="psB", bufs=4, space="PSUM") as psB, \
                 tc.tile_pool(name="actB", bufs=4) as actB:
                # DMA queue layout (A/B-tested): the gate set — xa, wg0,
                # wu0, exactly what the first matmuls need — leads both
                # HWDGE rings; everything else queues strictly behind on
                # the same rings (FIFO within a ring is real, and the 16
                # SDMA engines round-robin across queues, so an idle
                # queue firing early would steal HBM from the gate):
                #   ACT ring: xa, wu0, wu1, ..., wu7, (phase-D youts)
                #   SP ring:  wg0, wg1, xb, wg2, wg3, wd0, wg4, wd1, ...
                x_t = []
                x_srcs = []
                for ti, (t0, tw) in enumerate(ttiles):
                    t = xp.tile([KP, KH * tw], sdt, tag=f"x{ti}",
                                name=f"x{ti}")
                    src = xg[:, KH * t0:KH * (t0 + tw)]
                    if ti == 0:
                        nc.scalar.dma_start(out=t, in_=src)
                    else:
                        x_srcs.append((t, src))
                    x_t.append(t)

                # dummy matmuls: un-throttle the PE HAM to 2.4 GHz while
                # the first x/weight DMAs are in flight
                wsrc = actB.tile([KP, 512], mybir.dt.bfloat16,
                                 tag="wsrc", name="wsrc")
                nc.vector.memset(wsrc, 0.0)
                wdst = psB.tile([KP, 512], f32, tag="g", name="wdst")
                for w in range(DUMMIES):
                    nc.tensor.matmul(wdst, wsrc[:, :128], wsrc,
                                     start=(w == 0), stop=(w == DUMMIES - 1))

                # streamed column-grouped weight loads (shared-tag pool,
                # ONE contiguous 2-D DMA per group per matrix): wg on
                # the SP ring, wu on the ACT ring, consumption order
                wg_t, wu_t = {}, {}

                def load_w_group(q, eng_g=None, eng_u=None):
                    gn = W_GROUPS[q]
                    cw = gn * KP
                    o = KH * W_GSTART[q] * KP
                    t = wst.tile([KP, KH * cw], sdt, tag=f"w{gn}",
                                 bufs=W_BUFS[gn], name=f"wg{q}")
                    (eng_g or nc.sync).dma_start(
                        out=t, in_=wg[:, o:o + KH * cw])
                    wg_t[q] = t
                    t = wst.tile([KP, KH * cw], sdt, tag=f"w{gn}",
                                 bufs=W_BUFS[gn], name=f"wu{q}")
                    (eng_u or nc.scalar).dma_start(
                        out=t, in_=wu[:, o:o + KH * cw])
                    wu_t[q] = t

                for q in range(min(2, NG)):
                    load_w_group(q)
                # xb queues behind wg0/wg1 on the SP ring
                for t, src in x_srcs:
                    nc.sync.dma_start(out=t, in_=src)

                # wd chunks interleave into the SP ring at these i's —
                # behind the weight groups, through phase B's HBM slack
                wd_at = {1: 0, 2: 1, 4: 2, 6: 3, 10: 4, 12: 5}

                for i in range(KI):
                    q, r = i2q[i]
                    cw = W_GROUPS[q] * KP
                    if r == 0 and q + 2 < NG:
                        load_w_group(q + 2)
                    if i in wd_at:
                        load_wd_chunk(wd_at[i])
                    for ti, (t0, tw) in enumerate(ttiles):
                        g_ps = psB.tile([KP, tw], f32, tag="g",
                                        name=f"g{i}_{t0}")
                        u_ps = psB.tile([KP, tw], f32, tag="u",
                                        name=f"u{i}_{t0}")
                        for k in range(KH):
                            nc.tensor.matmul(
                                g_ps,
                                wg_t[q][:, k * cw + r * KP:
                                        k * cw + (r + 1) * KP],
                                x_t[ti][:, k * tw:(k + 1) * tw],
                                start=(k == 0), stop=(k == KH - 1))
                        for k in range(KH):
                            nc.tensor.matmul(
                                u_ps,
                                wu_t[q][:, k * cw + r * KP:
                                        k * cw + (r + 1) * KP],
                                x_t[ti][:, k * tw:(k + 1) * tw],
                                start=(k == 0), stop=(k == KH - 1))
                        sg = actB.tile([KP, tw], f32, tag="sg",
                                       name=f"sg{i}_{t0}")
                        nc.scalar.activation(
                            sg, g_ps, mybir.ActivationFunctionType.Silu)
                        nc.vector.tensor_mul(
                            h_t[i][:, t0:t0 + tw], sg, u_ps)

            # ---- phase D: yT = h @ wd, output [H, C]
            # First token tile: i-outer, streaming wd chunks just-in-time.
            # Later token tiles: hh-outer, reusing the resident wd tiles —
            # each output block finishes far apart, so the final
            # copies/stores are fully staggered and the kernel tail is
            # short.
            with tc.tile_pool(name="yout", bufs=4) as yp, \
                 tc.tile_pool(name="psD", bufs=1, space="PSUM") as psD:

                def emit_out(hh, t0, tw, y_ps):
                    yo = yp.tile([KP, tw], f32, tag="yo",
                                 name=f"yo{hh}_{t0}")
                    nc.vector.tensor_copy(yo, y_ps[hh])
                    nc.scalar.dma_start(
                        out=yT[hh * KP:(hh + 1) * KP, t0:t0 + tw],
                        in_=yo)

                for ti, (t0, tw) in enumerate(ttiles):
                    y_ps = [psD.tile([KP, tw], f32, tag=f"y{hh}",
                                     name=f"y{hh}_{t0}")
                            for hh in range(HH)]
                    if ti == 0:
                        for i in range(KI):
                            ci, io = i2c[i]
                            for hh in range(HH):
                                nc.tensor.matmul(
                                    y_ps[hh],
                                    wd_t[ci][:, io * H + hh * KP:
                                             io * H + (hh + 1) * KP],
                                    h_t[i][:, t0:t0 + tw],
                                    start=(i == 0), stop=(i == KI - 1))
                        for hh in range(HH):
                            emit_out(hh, t0, tw, y_ps)
                    else:
                        for hh in range(HH):
                            for i in range(KI):
                                ci, io = i2c[i]
                                nc.tensor.matmul(
                                    y_ps[hh],
                                    wd_t[ci][:, io * H + hh * KP:
                                             io * H + (hh + 1) * KP],
                                    h_t[i][:, t0:t0 + tw],
                                    start=(i == 0), stop=(i == KI - 1))
                            emit_out(hh, t0, tw, y_ps)
    _split_multi_waits(nc)
    return nc


CMAX = 1024   # per-run token capacity (bounded by SBUF for the h tiles)


def _pack_w(a, nblk, groups=None):
    """[nblk*128, N] fp32 -> [128, nblk*N] bf16, group-major.

    With groups (list of i-tile counts over N//128 column tiles), layout is
    [q][k][c] so each group is one contiguous per-partition run; without,
    layout is [i][c] (i-major, for wd chunking).
    """
    n = a.shape[1]
    blocks = a.reshape(nblk, KP, n)          # (k, p, c)
    if groups is None:
        # wd: i-major -> [128, nblk*N]
        return np.ascontiguousarray(
            blocks.transpose(1, 0, 2)).reshape(KP, nblk * n).astype(BF16)
    parts = []
    for q, gn in enumerate(groups):
        c0 = sum(groups[:q]) * KP
        cw = gn * KP
        # (k, p, cw) -> (p, k*cw)
        parts.append(blocks[:, :, c0:c0 + cw].transpose(1, 0, 2)
                     .reshape(KP, nblk * cw))
    return np.ascontiguousarray(np.concatenate(parts, axis=1)).astype(BF16)


def _fold(inputs):
    gp = np.asarray(inputs["gate_proj"], dtype=np.float32)
    up = np.asarray(inputs["up_proj"], dtype=np.float32)
    dp = np.asarray(inputs["down_proj"], dtype=np.float32)
    gA = np.asarray(inputs["gate_A"], dtype=np.float32)
    gB = np.asarray(inputs["gate_B"], dtype=np.float32)
    uA = np.asarray(inputs["up_A"], dtype=np.float32)
    uB = np.asarray(inputs["up_B"], dtype=np.float32)
    dA = np.asarray(inputs["down_A"], dtype=np.float32)
    dB = np.asarray(inputs["down_B"], dtype=np.float32)

    wmaps = []
    for e in range(E):
        wge = gp[e] + SCALING * (gA[e] @ gB[e])
        wue = up[e] + SCALING * (uA[e] @ uB[e])
        wde = dp[e] + SCALING * (dA[e] @ dB[e])
        wmaps.append({"wg": _pack_w(wge, KH, W_GROUPS),
                      "wu": _pack_w(wue, KH, W_GROUPS),
                      "wd": _pack_w(wde, KI)})
    return wmaps


def _route(inputs):
    hs = np.asarray(inputs["hidden_states"], dtype=np.float32)
    rw = np.asarray(inputs["routing_weights"], dtype=np.float32)
    se = np.asarray(inputs["selected_experts"]).astype(np.int64)
    T = hs.shape[0]

    combine = np.zeros((T, E), dtype=np.float32)
    for k in range(se.shape[1]):
        np.add.at(combine, (np.arange(T), se[:, k]), rw[:, k])

    idx = [np.nonzero(combine[:, e])[0] for e in range(E)]
    wts = [combine[idx[e], e] for e in range(E)]
    maxn = max((len(ix) for ix in idx), default=1)
    C = min(max(NTOK + 8, -(-maxn // 8) * 8), CMAX)
    return hs, idx, wts, C, maxn


def kernel(**inputs):
    _setup_paths()
    from concourse.bass_utils import run_bass_kernel_spmd

    hs, idx, wts, C, maxn = _route(inputs)
    wmaps = _fold(inputs)
    ttiles = _token_tiles(C)

    nc = _cache.get(C)
    if nc is None:
        nc = _build(C)
        _cache[C] = nc

    T = hs.shape[0]
    out = np.zeros((T, H), dtype=np.float32)
    nruns = max(1, -(-maxn // C))
    for r in range(nruns):
        in_maps = []
        for e in range(E):
            sub = idx[e][r * C:(r + 1) * C]
            # [n, H] -> [H, C] -> per token tile [k][t] contiguous
            xT = np.zeros((H, C), dtype=BF16)
            if len(sub):
                xT[:, :len(sub)] = hs[sub].T.astype(BF16)
            xb = xT.reshape(KH, KP, C)       # (k, p, t)
            xge = np.concatenate(
                [xb[:, :, t0:t0 + tw].transpose(1, 0, 2).reshape(KP, KH * tw)
                 for (t0, tw) in ttiles], axis=1)
            in_maps.append({"xg": np.ascontiguousarray(xge), **wmaps[e]})
        try:
            res = run_bass_kernel_spmd(
                nc, in_maps, core_ids=list(range(NCORES)))
        except Exception:
            import time
            time.sleep(2.0)
            res = run_bass_kernel_spmd(
                nc, in_maps, core_ids=list(range(NCORES)))

        # expose for external profiling harnesses (test.py)
        kernel._last = {"nc": nc, "in_maps": in_maps, "results": res}

        for e in range(E):
            sub = idx[e][r * C:(r + 1) * C]
            if not len(sub):
                continue
            w = wts[e][r * C:(r + 1) * C]
            yTe = res.results[e]["yT"]          # [H, C] fp32
            out[sub] += w[:, None] * yTe[:, :len(sub)].T
    return out
